# revision 1
# baseline (speedup 1.0000x reference)
"""Trainium2 Bass kernel for nn_MultiHeadFast (multi-head attention with
softmax over the QUERY axis).

Math (faithful to the reference):
  qkv = x @ Ws;  per (b,h):  S[q,k] = Q.K^T,  causal mask k<=q,
  P = softmax_over_q(S * T^-0.5),  out = P @ V.

Key layout trick: compute S TRANSPOSED (S^T[k,q], keys on partitions) so the
query-axis softmax is a free-axis reduction, and S^T is exactly the lhsT
operand needed for out^T = V^T @ P.  The normalizer (per key k) is folded
into V's rows before the PV matmul.  exp has no max-subtraction: |S*c| < 1.5.

Sharding: tensor-parallel over heads.  Core c owns heads {2c, 2c+1}; its Ws
column slice is passed from the host; no collectives.  Everything is bf16
with fp32 accumulation (measured ~5e-3 L2 error vs the fp32 reference).
"""

import numpy as np
from contextlib import ExitStack

import concourse.bass as bass
import concourse.mybir as mybir
import concourse.tile as tile
from concourse import bacc
from concourse.bass_utils import run_bass_kernel_spmd
from concourse.masks import make_identity

B, T, E = 2, 2048, 1024
H, D = 16, 64
NCORES = 8
HPC = H // NCORES            # heads per core = 2
FPC = HPC * D                # feature cols per core per Q/K/V = 128
P = 128
NT = B * T                   # 4096 tokens total
EK = E // P                  # 8 contraction blocks for QKV
NSLAB = T // 512             # 4 query slabs per batch
KTILES = T // P              # 16 key tiles per batch
DT = mybir.dt.bfloat16
F32 = mybir.dt.float32
SCALE = float(T) ** -0.5
NEG = -1e30


def build_kernel():
    nc = bacc.Bacc("TRN2", target_bir_lowering=False, debug=False)
    x_dram = nc.dram_tensor("x", (NT, E), F32, kind="ExternalInput")
    w_dram = nc.dram_tensor("wsl", (E, 3 * FPC), F32, kind="ExternalInput")
    out_dram = nc.dram_tensor("out", (B, T, FPC), F32, kind="ExternalOutput")

    with tile.TileContext(nc) as tc, ExitStack() as ctx:
        const = ctx.enter_context(tc.tile_pool(name="const", bufs=1))
        xtp = ctx.enter_context(tc.tile_pool(name="xtp", bufs=1))
        qkvp = ctx.enter_context(tc.tile_pool(name="qkvp", bufs=1))
        work = ctx.enter_context(tc.tile_pool(name="work", bufs=2))
        strips = ctx.enter_context(tc.tile_pool(name="strips", bufs=4))
        small = ctx.enter_context(tc.tile_pool(name="small", bufs=8))
        outp = ctx.enter_context(tc.tile_pool(name="outp", bufs=3))
        ps = ctx.enter_context(tc.tile_pool(name="ps", bufs=2, space="PSUM"))
        dram = ctx.enter_context(tc.tile_pool(name="dram", bufs=1, space="DRAM"))

        # ---- constants ----
        id_bf = const.tile([P, P], DT, name="id_bf")
        make_identity(nc, id_bf)
        id_f32 = const.tile([P, P], F32, name="id_f32")
        make_identity(nc, id_f32)
        zeros_bf = const.tile([P, P], DT, name="zeros_bf")
        nc.gpsimd.memset(zeros_bf[:], 0.0)
        # diagmask[p, f] = 0 if f >= p else NEG   (keys on partitions, q free)
        diagmask = const.tile([P, P], F32, name="diagmask")
        nc.gpsimd.memset(diagmask[:], 0.0)
        nc.gpsimd.affine_select(
            out=diagmask[:],
            in_=diagmask[:],
            compare_op=mybir.AluOpType.is_ge,
            fill=NEG,
            base=0,
            pattern=[[1, P]],
            channel_multiplier=-1,
        )

        # ---- phase A: x^T (bf16) via cast-DMA + DMA transpose ----
        # phase A strategy: load x fp32 natural (the only input DMA, 16MB),
        # cast to bf16 on GpSimd, transpose 128x128 blocks on the PE, and
        # interleave the QKV matmuls per 512-token slab as x^T becomes ready.
        wsl_f32 = qkvp.tile([P, EK, 3 * FPC], F32, name="wsl_f32")
        nc.sync.dma_start(wsl_f32[:], w_dram.rearrange("(eo ei) f -> ei eo f", ei=P))
        wsl = qkvp.tile([P, EK, 3 * FPC], DT, name="wsl")
        nc.vector.tensor_copy(wsl[:], wsl_f32[:])

        xT = xtp.tile([P, EK, NT], DT, name="xT")
        qt = qkvp.tile([P, NT], DT, name="qt")
        kt_sb = qkvp.tile([P, NT], DT, name="kt_sb")
        vt = qkvp.tile([P, NT], DT, name="vt")
        dsts = [qt, kt_sb, vt]
        for n in range(NT // 512):  # 512-token slabs
            xb = work.tile([P, 4, E], DT, tag="xb", bufs=2, name="xb")
            # SWDGE cast-DMA: fp32 DRAM -> bf16 SBUF, tokens on partitions
            nc.gpsimd.dma_start(
                out=xb[:],
                in_=x_dram[512 * n : 512 * (n + 1), :].rearrange(
                    "(w p) e -> p w e", p=P
                ),
            )
            for w in range(4):
                tp = ps.tile([P, E], DT, tag="pv", bufs=4, name="xtp")
                for e in range(EK):
                    nc.tensor.transpose(
                        tp[:, e * P : (e + 1) * P], xb[:, w, e * P : (e + 1) * P], id_bf[:]
                    )
                cp = nc.scalar.copy if w % 2 == 0 else nc.vector.tensor_copy
                cp(
                    xT[:, :, n * 512 + w * P : n * 512 + (w + 1) * P],
                    tp.rearrange("p (e c) -> p e c", c=P),
                )
            for m in range(3):
                mm_ps = ps.tile([P, 512], F32, tag="pv", bufs=4, name="qkv_ps")
                for e in range(EK):
                    nc.tensor.matmul(
                        mm_ps[:],
                        lhsT=wsl[:, e, m * P : (m + 1) * P],
                        rhs=xT[:, e, n * 512 : (n + 1) * 512],
                        start=(e == 0),
                        stop=(e == EK - 1),
                    )
                nc.scalar.copy(dsts[m][:, n * 512 : (n + 1) * 512], mm_ps[:])

        # ---- phase C: V^T -> V (tokens on partitions), per (b, hh) ----
        v_nat = qkvp.tile([P, B * HPC, KTILES, D], DT, name="v_nat")
        for b in range(B):
            for hh in range(HPC):
                for k in range(KTILES):
                    tok0 = b * T + k * P
                    tps = ps.tile([P, D], DT, tag="pv", bufs=4, name="vtp")
                    nc.tensor.transpose(
                        tps[:],
                        vt[hh * D : (hh + 1) * D, tok0 : tok0 + P],
                        id_bf[hh * D : (hh + 1) * D, hh * D : (hh + 1) * D],
                    )
                    nc.vector.tensor_copy(v_nat[:, b * HPC + hh, k, :], tps[:])

        # ---- phase D: attention per batch (software-pipelined over k) ----
        for b in range(B):
            pv_ps = [
                ps.tile([P, 512], F32, tag="pv", bufs=4, name=f"pv_{b}_{j}")
                for j in range(NSLAB)
            ]
            # Zero-initialize each PV accumulator bank with a full-width
            # zero matmul so every partition row's has_written state is set
            # identically under both the per-row and whole-bank semantics;
            # all real PV matmuls then accumulate with start=False.
            for j in range(NSLAB):
                nc.tensor.matmul(
                    pv_ps[j][:],
                    lhsT=zeros_bf[:],
                    rhs=qt[:, b * T : b * T + 512],
                    start=True,
                    stop=False,
                    skip_group_check=True,
                )

            def chunk_mms(b, k, hh, strip, coff, cw):
                """S^T matmuls + mask + exp for one chunk of a head strip."""
                j0 = k // 4
                q0 = 512 * j0
                dead = P * k - q0
                sps = ps.tile([P, 1024], F32, tag="sps", bufs=2, name="sps")
                for so in range(0, cw, 512):
                    qs = q0 + coff + so
                    nc.tensor.matmul(
                        sps[:, so : so + 512],
                        lhsT=kt_sb[hh * D : (hh + 1) * D, b * T + k * P : b * T + k * P + P],
                        rhs=qt[hh * D : (hh + 1) * D, b * T + qs : b * T + qs + 512],
                        start=True,
                        stop=True,
                    )
                acc = small.tile([P, 1], F32, tag="acc", name="acc")
                if coff == 0:
                    nc.vector.tensor_add(
                        sps[:, dead : dead + P], sps[:, dead : dead + P], diagmask[:]
                    )
                    if dead > 0:
                        nc.gpsimd.memset(strip[:, 0:dead], 0.0)
                    nc.scalar.activation(
                        strip[:, dead:cw],
                        sps[:, dead:cw],
                        mybir.ActivationFunctionType.Exp,
                        scale=SCALE,
                        accum_out=acc[:],
                    )
                else:
                    nc.scalar.activation(
                        strip[:, coff : coff + cw],
                        sps[:, :cw],
                        mybir.ActivationFunctionType.Exp,
                        scale=SCALE,
                        accum_out=acc[:],
                    )
                return acc

            def finish_head(b, k, hh, partials):
                if len(partials) == 1:
                    ssum = partials[0]
                else:
                    ssum = small.tile([P, 1], F32, tag="acc", name="ssum")
                    nc.vector.tensor_add(ssum[:], partials[0][:], partials[1][:])
                rsum = small.tile([P, 1], F32, tag="acc", name="rsum")
                nc.vector.reciprocal(rsum[:], ssum[:])
                vp = small.tile([P, D], DT, tag="vp", name="vp")
                nc.vector.tensor_scalar_mul(
                    vp[:], v_nat[:, b * HPC + hh, k, :], rsum[:]
                )
                return vp

            def pv_head(b, k, hh, strip, vp):
                j0 = k // 4
                q0 = 512 * j0
                for j in range(j0, NSLAB):
                    nc.tensor.matmul(
                        pv_ps[j][hh * D : (hh + 1) * D, :],
                        lhsT=vp[:],
                        rhs=strip[:, 512 * j - q0 : 512 * j - q0 + 512],
                        start=False,
                        stop=(k == 4 * j + 3 and hh == HPC - 1),
                        skip_group_check=True,
                    )

            # software pipeline: chunk-level head alternation keeps 2 chunks
            # in flight (one per head) so the ACT exp stream never starves;
            # PV matmuls of k-1 fill the PE between chunk groups.
            prev = {}
            for k in range(KTILES):
                j0 = k // 4
                L = T - 512 * j0
                strip_k = {}
                parts = {0: [], 1: []}
                for hh in range(HPC):
                    strip_k[hh] = strips.tile([P, T], DT, tag="strip", name=f"s{hh}")
                coff = 0
                while coff < L:
                    cw = min(1024, L - coff)
                    for hh in range(HPC):
                        parts[hh].append(chunk_mms(b, k, hh, strip_k[hh], coff, cw))
                    coff += cw
                for hh in range(HPC):
                    vp = finish_head(b, k, hh, parts[hh])
                    if k > 0:
                        pv_head(b, k - 1, hh, *prev[hh])
                    prev[hh] = (strip_k[hh], vp)
            for hh in range(HPC):
                pv_head(b, KTILES - 1, hh, *prev[hh])
            # evacuate + transpose out^T -> out
            for j in range(NSLAB):
                osb = outp.tile([P, 512], F32, tag="osb", name="osb")
                nc.vector.tensor_copy(osb[:], pv_ps[j][:])
                o_sb = outp.tile([P, 4, P], F32, tag="o_sb", name="o_sb")
                for w in range(4):
                    tp = ps.tile([P, P], F32, tag="pv", bufs=4, name="otp")
                    nc.tensor.transpose(tp[:], osb[:, w * P : (w + 1) * P], id_f32[:])
                    nc.vector.tensor_copy(o_sb[:, w, :], tp[:])
                nc.sync.dma_start(
                    out_dram[b, 512 * j : 512 * (j + 1), :].rearrange(
                        "(w p) f -> p w f", p=P
                    ),
                    o_sb[:],
                )
    nc.compile()
    return nc


_NC_CACHE = None


def kernel(x: np.ndarray, Ws: np.ndarray) -> np.ndarray:
    global _NC_CACHE
    if _NC_CACHE is None:
        _NC_CACHE = build_kernel()
    nc = _NC_CACHE

    x2 = np.ascontiguousarray(x.reshape(NT, E).astype(np.float32, copy=False))
    in_maps = []
    for c in range(NCORES):
        cols = np.concatenate(
            [
                Ws[:, c * FPC : (c + 1) * FPC],
                Ws[:, E + c * FPC : E + (c + 1) * FPC],
                Ws[:, 2 * E + c * FPC : 2 * E + (c + 1) * FPC],
            ],
            axis=1,
        ).astype(np.float32, copy=False)
        in_maps.append({"x": x2, "wsl": np.ascontiguousarray(cols)})

    res = run_bass_kernel_spmd(nc, in_maps, core_ids=list(range(NCORES)))
    out = np.empty((B, T, H * D), np.float32)
    for c in range(NCORES):
        out[:, :, c * FPC : (c + 1) * FPC] = res.results[c]["out"]
    return out



# revision 5
# speedup vs baseline: 1.2728x; 1.2728x over previous
"""Trainium2 Bass kernel for nn_MultiHeadFast (multi-head attention with
softmax over the QUERY axis).

Math (faithful to the reference):
  qkv = x @ Ws;  per (b,h):  S[q,k] = Q.K^T,  causal mask k<=q,
  P = softmax_over_q(S * T^-0.5),  out = P @ V.

v2 design (PE-minimal):
  - Sharding: 8 cores = 2 batches x 4 head-groups.  Core c owns batch c//4
    and 4 heads, processed as 2 passes of 2 heads (PSUM limit).
  - Host passes x^T (bf16) and the per-core Ws column slice (bf16), so the
    device does ZERO input transposes.  V is produced token-major via an
    XBAR DMA transpose (out[p,i,d] = in[d, i*128+p]), not the PE.
  - S^T is computed only on the live causal region (q >= 128*ktile) in
    <=512-col matmuls; exp (query-axis softmax numerator) on ScalarE;
    per-key normalizers via VectorE free-axis reduce of the bf16 strip.
  - out^T[d,q] = sum_k V_norm^T P^T accumulates in PSUM and is DMA'd out
    transposed; the host does the final cheap (128,2048)->(2048,128)
    transpose.  Pipeline: [QKV pass0] [S/exp pass0 + QKV pass1 on PE]
    [S/exp pass1 + PV pass0 + PV pass1 slab-major] [PV1 tail].
"""

import numpy as np
import ml_dtypes
from contextlib import ExitStack

import concourse.bass as bass
import concourse.mybir as mybir
import concourse.tile as tile
from concourse import bacc
from concourse.bass_utils import run_bass_kernel_spmd

B, T, E = 2, 2048, 1024
H, D = 16, 64
NCORES = 8
P = 128
EK = E // P           # 8 contraction blocks
KT = T // P           # 16 key tiles per core-batch
NS = T // 512         # 4 query slabs
DT = mybir.dt.bfloat16
F32 = mybir.dt.float32
SCALE = float(T) ** -0.5
NEG = -1e30


def live(k):
    return T - P * k


def build_kernel():
    nc = bacc.Bacc("TRN2", target_bir_lowering=False, debug=False)
    x_dram = nc.dram_tensor("x", (E, T), DT, kind="ExternalInput")       # x^T
    w_dram = nc.dram_tensor("wsl", (E, 768), DT, kind="ExternalInput")
    out_dram = nc.dram_tensor("out", (2, P, T), F32, kind="ExternalOutput")

    with tile.TileContext(nc) as tc, ExitStack() as ctx:
        const = ctx.enter_context(tc.tile_pool(name="const", bufs=1))
        xp = ctx.enter_context(tc.tile_pool(name="xp", bufs=1))
        qkvp = ctx.enter_context(tc.tile_pool(name="qkvp", bufs=1))
        strips = ctx.enter_context(tc.tile_pool(name="strips", bufs=1))
        small = ctx.enter_context(tc.tile_pool(name="small", bufs=1))
        ps = ctx.enter_context(tc.tile_pool(name="ps", bufs=1, space="PSUM"))

        # ---- constants ----
        zeros_bf = const.tile([P, P], DT, name="zeros_bf")
        nc.gpsimd.memset(zeros_bf[:], 0.0)
        # diagmask[p, f] = 0 if f >= p else NEG (keys on partitions, q free)
        diagmask = const.tile([P, P], F32, name="diagmask")
        nc.gpsimd.memset(diagmask[:], 0.0)
        nc.gpsimd.affine_select(
            out=diagmask[:],
            in_=diagmask[:],
            compare_op=mybir.AluOpType.is_ge,
            fill=NEG,
            base=0,
            pattern=[[1, P]],
            channel_multiplier=-1,
        )

        # ---- input DMAs ----
        wsl = qkvp.tile([P, EK, 768], DT, name="wsl")
        nc.sync.dma_start(wsl[:], w_dram.rearrange("(eo ei) f -> ei eo f", ei=P))
        xT = xp.tile([P, EK, T], DT, name="xT")
        for s in range(NS):
            nc.sync.dma_start(
                xT[:, :, 512 * s : 512 * (s + 1)],
                x_dram[:, 512 * s : 512 * (s + 1)].rearrange(
                    "(eo ei) t -> ei eo t", ei=P
                ),
            )

        # ---- per-pass tensors ----
        qt = [qkvp.tile([P, T], DT, name=f"qt{p}") for p in range(2)]
        kt = [qkvp.tile([P, T], DT, name=f"kt{p}") for p in range(2)]
        vt = [qkvp.tile([P, T], DT, name=f"vt{p}") for p in range(2)]
        vnat = [qkvp.tile([P, KT, P], DT, name=f"vn{p}") for p in range(2)]
        vp_all = [qkvp.tile([P, KT, 2, D], DT, name=f"vp{p}") for p in range(2)]

        def qkv_unit(p, m, s):
            """One 512-token slab of Q^T/K^T/V^T (m=0/1/2) for pass p."""
            dst = (qt, kt, vt)[m][p]
            mm = ps.tile([P, 512], F32, tag="b512", bufs=4, name="qkv_ps")
            for e in range(EK):
                nc.tensor.matmul(
                    mm[:],
                    lhsT=wsl[:, e, 256 * m + P * p : 256 * m + P * (p + 1)],
                    rhs=xT[:, e, 512 * s : 512 * (s + 1)],
                    start=(e == 0),
                    stop=(e == EK - 1),
                )
            nc.vector.tensor_copy(dst[:, 512 * s : 512 * (s + 1)], mm[:])

        def s_exp_unit(p, k, h, strip):
            """S^T matmuls + mask + exp + normalizer for (pass, ktile, head)."""
            L = live(k)
            q0 = P * k
            for c in range(0, L, 1024):
                cw = min(1024, L - c)
                sps = ps.tile([P, 1024], F32, tag="sps", bufs=2, name="sps")
                for so in range(0, cw, 512):
                    w = min(512, cw - so)
                    nc.tensor.matmul(
                        sps[:, so : so + w],
                        lhsT=kt[p][h * D : (h + 1) * D, q0 : q0 + P],
                        rhs=qt[p][h * D : (h + 1) * D, q0 + c + so : q0 + c + so + w],
                        start=True,
                        stop=True,
                    )
                if c == 0:
                    nc.vector.tensor_add(sps[:, 0:P], sps[:, 0:P], diagmask[:])
                nc.scalar.activation(
                    strip[:, c : c + cw],
                    sps[:, :cw],
                    mybir.ActivationFunctionType.Exp,
                    scale=SCALE,
                )
            ssum = small.tile([P, 1], F32, tag="acc", bufs=4, name="ssum")
            nc.vector.tensor_reduce(
                ssum[:], strip[:], axis=mybir.AxisListType.X, op=mybir.AluOpType.add
            )
            rsum = small.tile([P, 1], F32, tag="acc", bufs=4, name="rsum")
            nc.vector.reciprocal(rsum[:], ssum[:])
            nc.vector.tensor_scalar_mul(
                vp_all[p][:, k, h, :], vnat[p][:, k, :][:, h * D : (h + 1) * D], rsum[:]
            )

        def pv_mms(p, k, h, strip, pv, j, last):
            """PV contribution of (pass, ktile, head) to out^T slab j."""
            j0 = k // 4
            if j == j0:
                coff = P * (k % 4)
                nc.tensor.matmul(
                    pv[h * D : (h + 1) * D, coff:512],
                    lhsT=vp_all[p][:, k, h, :],
                    rhs=strip[:, 0 : 512 - coff],
                    start=False,
                    stop=last,
                    skip_group_check=True,
                )
            else:
                c = 512 * j - P * k
                nc.tensor.matmul(
                    pv[h * D : (h + 1) * D, :],
                    lhsT=vp_all[p][:, k, h, :],
                    rhs=strip[:, c : c + 512],
                    start=False,
                    stop=last,
                    skip_group_check=True,
                )

        def dma_out_slab(p, j, pv):
            ob = strips.tile([P, 512], F32, tag="outb", bufs=2, name="outb")
            nc.vector.tensor_copy(ob[:], pv[:])
            nc.sync.dma_start(out_dram[p, :, 512 * j : 512 * (j + 1)], ob[:])

        def new_pv_bank():
            pv = ps.tile([P, 512], F32, tag="b512", bufs=4, name="pv")
            nc.tensor.matmul(
                pv[:],
                lhsT=zeros_bf[:],
                rhs=xT[:, 0, 0:512],
                start=True,
                stop=False,
                skip_group_check=True,
            )
            return pv

        # ================= seg A: QKV pass 0 =================
        for s in range(NS):
            for m in range(3):
                qkv_unit(0, m, s)
        nc.sync.dma_start_transpose(vnat[0][:], vt[0][:])

        # ========= seg B: attention pass 0 (S/exp) + QKV pass 1 =========
        strip0 = {}
        for k in range(KT):
            for h in range(2):
                st = strips.tile([P, live(k)], DT, tag=f"s{k}", bufs=2, name=f"s{k}")
                s_exp_unit(0, k, h, st)
                strip0[(k, h)] = st
            if k < 12:
                qkv_unit(1, k // 4, k % 4)
        nc.sync.dma_start_transpose(vnat[1][:], vt[1][:])

        # ==== seg C: S/exp pass 1 + PV pass 0 + PV pass 1 slab-major ====
        strip1 = {}
        pv0 = {}
        pv1 = {}

        def emit_pv1_slab(j):
            pv1[j] = new_pv_bank()
            for kk in range(4 * j + 4):
                for h in range(2):
                    pv_mms(1, kk, h, strip1[(kk, h)], pv1[j], j,
                           last=(kk == 4 * j + 3 and h == 1))
            dma_out_slab(1, j, pv1[j])

        for j in range(NS):
            pv0[j] = new_pv_bank()
        for k in range(KT):
            j0 = k // 4
            for h in range(2):
                for j in range(j0, NS):
                    pv_mms(0, k, h, strip0[(k, h)], pv0[j], j,
                           last=(k == 4 * j + 3 and h == 1))
            if k % 4 == 3:
                dma_out_slab(0, j0, pv0[j0])
            for h in range(2):
                st = strips.tile([P, live(k)], DT, tag=f"s{k}", bufs=2, name=f"t{k}")
                s_exp_unit(1, k, h, st)
                strip1[(k, h)] = st
            if k >= 4 and k % 4 == 0:
                emit_pv1_slab(k // 4 - 1)

        # ================= seg D: PV pass 1 tail =================
        emit_pv1_slab(3)

    nc.compile()
    return nc


def prep_in_maps(x, Ws):
    x = np.asarray(x, np.float32)
    Ws = np.asarray(Ws, np.float32)
    in_maps = []
    for c in range(NCORES):
        b = c // 4
        xT = np.ascontiguousarray(x[b].T).astype(ml_dtypes.bfloat16)
        blocks = []
        for m in range(3):
            for p in range(2):
                g = (c % 4) * 4 + 2 * p
                blocks.append(Ws[:, m * E + D * g : m * E + D * g + 2 * D])
        wsl = np.concatenate(blocks, axis=1).astype(ml_dtypes.bfloat16)
        in_maps.append({"x": xT, "wsl": np.ascontiguousarray(wsl)})
    return in_maps


def assemble(results):
    out = np.empty((B, T, H * D), np.float32)
    for c in range(NCORES):
        r = np.asarray(results[c]["out"], np.float32)
        b = c // 4
        for p in range(2):
            for h in range(2):
                g = (c % 4) * 4 + 2 * p + h
                out[b, :, D * g : D * (g + 1)] = r[p, D * h : D * (h + 1), :].T
    return out


_NC_CACHE = None


def kernel(x: np.ndarray, Ws: np.ndarray) -> np.ndarray:
    global _NC_CACHE
    if _NC_CACHE is None:
        _NC_CACHE = build_kernel()
    nc = _NC_CACHE
    res = run_bass_kernel_spmd(nc, prep_in_maps(x, Ws), core_ids=list(range(NCORES)))
    return assemble(res.results)
